# revision 20
# baseline (speedup 1.0000x reference)
"""Trainium2 Bass kernel for nn_AdaptiveKernelModule (dense_cnn).

Math: the per-sample dynamic conv kernel is rank-2 in its output channel:
    gk[o,i,kh,kw] = Wk[o] * g[i,kh,kw] + bk[o]
so with u = Wf@Wk, v = Wf@bk, w = Wf@b_adap + bf (host-precomputed):
    out[c, p] = u[c] * A[p] + v[c] * B[p] + w[c]
    A[p] = sum_{i,kh,kw} g[i,kh,kw] * f[i, p + delta(kh,kw)]
    B[p] = sum_{i,kh,kw}              f[i, p + delta(kh,kw)]
    f    = relu(W1 @ x + b1)

Device pipeline per sample (2 samples per core, 8 cores data-parallel over N):
  x arrives bf16 (host-converted; MM1 consumes bf16 anyway, identical result)
  MM1: 8 chunks of f_psum = W1T.T @ x_chunk (bf16, K=128, M=32) into one
       [128, 2x512] PSUM pair; one relu+b1 evac per x-tile (ACT/DVE alternate)
       writes the row-interleaved f_pad (partition group g=(row%8)//2)
  maxpool 64x64 on DVE, 2-stage (X-reduce then strided combine) -> xp;
       tiny MM (replicated W1T, M=128) + relu -> G2 tap columns
  MM2: T_psum = G2[32g].T @ f_pad block (bf16, K=32, M=10 at psum rows
       96..105), two chunks per PSUM pair, evac (DVE/ACT alternate) -> T_plain
  DMA SBUF->SBUF: T_sb[t, q] = T_plain[row(t), q + delta_t] (18 shifted rows,
       2 fixed 18818-elem chunks each; odd-length bf16 copies are pathological)
  MM3: out_psum = L3.T @ T_sb_chunk (bf16, K=18, M=128), two chunks per PSUM
       pair, bias evac (ACT/DVE alternate) -> f32 out tile -> DMA to HBM.
Large HBM DMAs are split into partition halves so two queues run per tile;
PSUM pools are split (MM1 vs MM2/MM3) so phases do not serialize each other.
"""

import numpy as np
import ml_dtypes

import concourse.bass as bass
import concourse.bacc as bacc
import concourse.mybir as mybir
import concourse.tile as tile
from concourse.bass_utils import run_bass_kernel_spmd

F32 = mybir.dt.float32
BF16 = mybir.dt.bfloat16

N_CORES = 8
NS = 2
C = 128
CM = 32
H = W = 192
HP = WP = 194
L = HP * WP
XROWS = 16
RROWS = 2
NB = H // 8

TSB = 64
TPL = 96
GCOL = 96

DELTAS = [(kh - 1) * WP + (kw - 1) for kh in range(3) for kw in range(3)]


def build(nc):
    x_d = nc.declare_dram_parameter("x", [NS, C, H, W], BF16, isOutput=False)
    w1t4_d = nc.declare_dram_parameter("w1t4", [C, C], BF16, isOutput=False)
    b14_d = nc.declare_dram_parameter("b14", [C, 1], F32, isOutput=False)
    l3_d = nc.declare_dram_parameter("l3", [18, C], BF16, isOutput=False)
    wb_d = nc.declare_dram_parameter("wb", [C, 1], F32, isOutput=False)
    out_d = nc.declare_dram_parameter("out", [NS, C, H, W], F32, isOutput=True)

    with tile.TileContext(nc) as tc:
        with (
            tc.tile_pool(name="persist", bufs=1) as pp,
            tc.tile_pool(name="xbf", bufs=4) as xbf_pool,
            tc.tile_pool(name="outp", bufs=3) as out_pool,
            tc.tile_pool(name="small", bufs=2) as sp,
            tc.tile_pool(name="psf", bufs=2, space="PSUM") as psf_pool,
            tc.tile_pool(name="pst", bufs=2, space="PSUM") as pst_pool,
        ):
            tmeg = pp.tile([128, L], BF16)
            LF = NB * 2 * WP
            f4a = pp.tile([128, LF], BF16)
            f4b = pp.tile([128, LF], BF16)
            w1t4_sb = pp.tile([C, C], BF16)
            b14_sb = pp.tile([C, 1], F32)
            l3_sb = pp.tile([128, C], BF16)
            wb_sb = pp.tile([C, 1], F32)
            g2 = pp.tile([128, C], BF16)

            nc.sync.dma_start(out=w1t4_sb[:, :], in_=w1t4_d.ap())
            nc.sync.dma_start(out=b14_sb[:, :], in_=b14_d.ap())
            nc.sync.dma_start(out=l3_sb[64:82, :], in_=l3_d.ap())
            nc.sync.dma_start(out=wb_sb[:, :], in_=wb_d.ap())

            nc.vector.memset(g2[:, :], 0.0)
            nc.vector.memset(g2[:, GCOL + 9 : GCOL + 10], 1.0)
            nc.vector.memset(tmeg[TPL : TPL + 10, 0:WP], 0.0)
            nc.vector.memset(tmeg[TPL : TPL + 10, 193 * WP : 194 * WP], 0.0)
            for f4 in (f4a, f4b):
                f4v = f4.rearrange("p (b r c) -> p b r c", r=2, c=WP)
                nc.vector.memset(f4v[:, :, :, 0:1], 0.0)
                nc.vector.memset(f4v[:, :, :, WP - 1 : WP], 0.0)

            for n in range(NS):
                f4 = f4a if n % 2 == 0 else f4b

                # -------- pass 1: x in (bf16, split DMA), maxpool, MM1+relu
                ntiles = H // XROWS  # 12
                xp_part = sp.tile([128, ntiles * 48], F32, tag="xp_part")
                for j in range(ntiles):
                    xb = xbf_pool.tile([128, XROWS * W], BF16, tag="xb")
                    xb3 = xb.rearrange("p (r c) -> p r c", c=W)
                    for hh in range(2):
                        nc.sync.dma_start(
                            out=xb3[64 * hh : 64 * hh + 64],
                            in_=x_d.ap()[
                                n, 64 * hh : 64 * hh + 64,
                                j * XROWS : (j + 1) * XROWS, :,
                            ],
                        )
                    # maxpool stage 1: X-reduce over contiguous 64 cols
                    nc.vector.tensor_reduce(
                        xp_part[:, j * 48 : (j + 1) * 48],
                        xb.rearrange("p (r kx c) -> p r kx c", kx=3, c=64),
                        axis=mybir.AxisListType.X,
                        op=mybir.AluOpType.max,
                    )
                    ps = psf_pool.tile([128, 1024], F32, tag="psf", name="psf")
                    for half in range(2):
                        for g in range(4):
                            yloc = half * 8 + 2 * g
                            nc.tensor.matmul(
                                ps[
                                    32 * g : 32 * g + 32,
                                    half * 512 : half * 512 + RROWS * W,
                                ],
                                w1t4_sb[:, 32 * g : 32 * g + 32],
                                xb[:, yloc * W : (yloc + 2) * W],
                                tile_position=(0, 32 * g),
                            )
                    dst = f4.rearrange("p (b r c) -> p b r c", r=2, c=WP)[
                        :, 2 * j : 2 * j + 2, :, 1 : 1 + W
                    ]
                    src = ps.rearrange("p (h f) -> p h f", h=2)[
                        :, :, : RROWS * W
                    ].rearrange("p h (r c) -> p h r c", c=W)
                    if j % 2 == 0:
                        nc.scalar.activation(
                            dst, src,
                            mybir.ActivationFunctionType.Relu,
                            bias=b14_sb[:, :],
                        )
                    else:
                        nc.vector.tensor_scalar(
                            dst, src, b14_sb[:, :], 0.0,
                            op0=mybir.AluOpType.add,
                            op1=mybir.AluOpType.max,
                        )

                # -------- maxpool stage 2 (two strided combines), then g
                xp_mid = sp.tile([128, 36], F32, tag="xp_mid")
                nc.vector.tensor_reduce(
                    xp_mid[:, :],
                    xp_part.rearrange("p (j r kx) -> p j kx r", r=16, kx=3),
                    axis=mybir.AxisListType.X,
                    op=mybir.AluOpType.max,
                )
                xp_r = sp.tile([128, 10], BF16, tag="xp_r")
                nc.vector.memset(xp_r[:, 9:10], 0.0)
                nc.vector.tensor_reduce(
                    xp_r[:, 0:9],
                    xp_mid.rearrange("p (ky s kx) -> p ky kx s", ky=3, kx=3),
                    axis=mybir.AxisListType.X,
                    op=mybir.AluOpType.max,
                )
                pg = pst_pool.tile([128, 1024], F32, tag="pst", name="pg")[:, :10]
                nc.tensor.matmul(pg[:, :], w1t4_sb[:, :], xp_r[:, :])
                nc.scalar.activation(
                    g2[:, GCOL : GCOL + 9],
                    pg[:, 0:9],
                    mybir.ActivationFunctionType.Relu,
                    bias=b14_sb[:, :],
                )

                # -------- MM2 / T-shift / MM3 in two half-plane stages so
                # stage-1 MM2 overlaps stage-0 MM3 (kills the phase valley)
                tpl = tmeg[TPL : TPL + 10, :]
                f4r = f4.rearrange("p (b f) -> p b f", f=2 * WP)
                tsb = tmeg[TSB : TSB + 18, :].rearrange("p (r c) -> p r c", c=WP)

                def mm2_block(b):
                    for gp in range(2):
                        pT = pst_pool.tile([128, 1024], F32, tag="pst", name="pT")
                        for q in range(2):
                            g = 2 * gp + q
                            nc.tensor.matmul(
                                pT[TPL : TPL + 10, q * 512 : q * 512 + 2 * WP],
                                g2[32 * g : 32 * g + 32, GCOL : GCOL + 10],
                                f4r[32 * g : 32 * g + 32, b, :],
                                tile_position=(32 * g, TPL),
                            )
                        py0 = 8 * b + 4 * gp + 1
                        dst = tpl[:, py0 * WP : (py0 + 4) * WP]
                        src = pT.rearrange("p (q f) -> p q f", q=2)[
                            TPL : TPL + 10, :, : 2 * WP
                        ]
                        nc.vector.tensor_copy(dst, src)

                def tshift(stage):
                    CH0, CH1 = 19040, 18820
                    for t in range(18):
                        src_row = TPL + (t if t < 9 else 9)
                        d = DELTAS[t % 9]
                        if stage == 0:
                            lo = max(0, -d)
                            hi = lo + CH0
                        else:
                            hi = L - max(0, d)
                            lo = hi - CH1
                        nc.scalar.dma_start(
                            out=tmeg[TSB + t : TSB + t + 1, lo:hi],
                            in_=tmeg[src_row : src_row + 1, lo + d : hi + d],
                        )

                def mm3_tile(j):
                    ot = out_pool.tile([128, XROWS * W], F32, tag="ot")
                    for half in range(4):
                        po = pst_pool.tile([128, 1024], F32, tag="pst", name="po")
                        for q in range(2):
                            y0 = j * XROWS + (half * 2 + q) * RROWS
                            nc.tensor.matmul(
                                po[:, q * 512 : q * 512 + RROWS * W],
                                l3_sb[64:82, :],
                                tsb[:, y0 + 1 : y0 + 1 + RROWS, 1 : 1 + W],
                            )
                        dst = ot[
                            :, half * 2 * RROWS * W : (half + 1) * 2 * RROWS * W
                        ]
                        src = po.rearrange("p (q f) -> p q f", q=2)[
                            :, :, : RROWS * W
                        ]
                        if (j + half) % 2 == 0:
                            nc.scalar.activation(
                                dst, src,
                                mybir.ActivationFunctionType.Identity,
                                bias=wb_sb[:, :],
                            )
                        else:
                            nc.vector.tensor_scalar(
                                dst, src, wb_sb[:, :], None,
                                op0=mybir.AluOpType.add,
                            )
                    ot3 = ot.rearrange("p (r c) -> p r c", c=W)
                    for hh in range(2):
                        nc.scalar.dma_start(
                            out=out_d.ap()[
                                n, 64 * hh : 64 * hh + 64,
                                j * XROWS : (j + 1) * XROWS, :,
                            ],
                            in_=ot3[64 * hh : 64 * hh + 64],
                        )

                for b in range(14):
                    mm2_block(b)
                tshift(0)
                for b in range(14, NB):
                    mm2_block(b)
                for j in range(ntiles // 2):
                    mm3_tile(j)
                tshift(1)
                for j in range(ntiles // 2, ntiles):
                    mm3_tile(j)
    return nc


_CACHE = {}


def _get_nc():
    if "nc" not in _CACHE:
        nc = bacc.Bacc(
            "TRN2", target_bir_lowering=False, debug=False, num_devices=N_CORES
        )
        build(nc)
        nc.compile()
        _CACHE["nc"] = nc
    return _CACHE["nc"]


def make_in_maps(x, W1, b1, Wk, bk, b_adap, Wf, bf):
    x = np.asarray(x, dtype=np.float32)
    W1 = np.asarray(W1, dtype=np.float32)
    b1 = np.asarray(b1, dtype=np.float32)
    Wk = np.asarray(Wk, dtype=np.float32)
    bk = np.asarray(bk, dtype=np.float32)
    b_adap = np.asarray(b_adap, dtype=np.float32)
    Wf = np.asarray(Wf, dtype=np.float32)
    bf = np.asarray(bf, dtype=np.float32)

    u = Wf @ Wk
    v = Wf @ bk
    w = Wf @ b_adap + bf
    l3 = np.ascontiguousarray(np.stack([u] * 9 + [v] * 9).astype(ml_dtypes.bfloat16))
    w1t4 = np.ascontiguousarray(np.tile(W1.T, (1, 4)).astype(ml_dtypes.bfloat16))
    b14 = np.ascontiguousarray(np.tile(b1, 4)[:, None].astype(np.float32))
    wbc = np.ascontiguousarray(w[:, None].astype(np.float32))
    xb = np.ascontiguousarray(x.astype(ml_dtypes.bfloat16))

    in_maps = []
    for i in range(N_CORES):
        in_maps.append(
            {
                "x": xb[i * NS : (i + 1) * NS],
                "w1t4": w1t4,
                "b14": b14,
                "l3": l3,
                "wb": wbc,
            }
        )
    return in_maps


def kernel(x, W1, b1, Wk, bk, b_adap, Wf, bf):
    nc = _get_nc()
    in_maps = make_in_maps(x, W1, b1, Wk, bk, b_adap, Wf, bf)
    res = run_bass_kernel_spmd(nc, in_maps, list(range(N_CORES)))
    return np.concatenate([res.results[i]["out"] for i in range(N_CORES)], axis=0)


# revision 21
# speedup vs baseline: 1.0433x; 1.0433x over previous
"""Trainium2 Bass kernel for nn_AdaptiveKernelModule (dense_cnn).

Math: the per-sample dynamic conv kernel is rank-2 in its output channel:
    gk[o,i,kh,kw] = Wk[o] * g[i,kh,kw] + bk[o]
so with u = Wf@Wk, v = Wf@bk, w = Wf@b_adap + bf (host-precomputed):
    out[c, p] = u[c] * A[p] + v[c] * B[p] + w[c]
    A[p] = sum_{i,kh,kw} g[i,kh,kw] * f[i, p + delta(kh,kw)]
    B[p] = sum_{i,kh,kw}              f[i, p + delta(kh,kw)]
    f    = relu(W1 @ x + b1)

Device pipeline per sample (2 samples per core, 8 cores data-parallel over N):
  x arrives bf16 (host-converted; MM1 consumes bf16 anyway, identical result)
  MM1: 8 chunks of f_psum = W1T.T @ x_chunk (bf16, K=128, M=32) into one
       [128, 2x512] PSUM pair; one relu+b1 evac per x-tile (ACT/DVE alternate)
       writes the row-interleaved f_pad (partition group g=(row%8)//2)
  maxpool 64x64 on DVE, 2-stage (X-reduce then strided combine) -> xp;
       tiny MM (replicated W1T, M=128) + relu -> G2 tap columns
  MM2: T_psum = G2[32g].T @ f_pad block (bf16, K=32, M=10 at psum rows
       96..105), two chunks per PSUM pair, evac (DVE/ACT alternate) -> T_plain
  DMA SBUF->SBUF: T_sb[t, q] = T_plain[row(t), q + delta_t] (18 shifted rows,
       2 fixed 18818-elem chunks each; odd-length bf16 copies are pathological)
  MM3: out_psum = L3.T @ T_sb_chunk (bf16, K=18, M=128), two chunks per PSUM
       pair, bias evac (ACT/DVE alternate) -> f32 out tile -> DMA to HBM.
Large HBM DMAs are split into partition halves so two queues run per tile;
PSUM pools are split (MM1 vs MM2/MM3) so phases do not serialize each other.
"""

import numpy as np
import ml_dtypes

import concourse.bass as bass
import concourse.bacc as bacc
import concourse.mybir as mybir
import concourse.tile as tile
from concourse.bass_utils import run_bass_kernel_spmd

F32 = mybir.dt.float32
BF16 = mybir.dt.bfloat16

N_CORES = 8
NS = 2
C = 128
CM = 32
H = W = 192
HP = WP = 194
L = HP * WP
XROWS = 16
RROWS = 2
NB = H // 8

TSB = 64
TPL = 96
GCOL = 96

DELTAS = [(kh - 1) * WP + (kw - 1) for kh in range(3) for kw in range(3)]


def build(nc):
    x_d = nc.declare_dram_parameter("x", [NS, C, H, W], BF16, isOutput=False)
    w1t4_d = nc.declare_dram_parameter("w1t4", [C, C], BF16, isOutput=False)
    b14_d = nc.declare_dram_parameter("b14", [C, 1], F32, isOutput=False)
    l3_d = nc.declare_dram_parameter("l3", [18, C], BF16, isOutput=False)
    wb_d = nc.declare_dram_parameter("wb", [C, 1], F32, isOutput=False)
    out_d = nc.declare_dram_parameter("out", [NS, C, H, W], BF16, isOutput=True)

    with tile.TileContext(nc) as tc:
        with (
            tc.tile_pool(name="persist", bufs=1) as pp,
            tc.tile_pool(name="xbf", bufs=4) as xbf_pool,
            tc.tile_pool(name="outp", bufs=3) as out_pool,
            tc.tile_pool(name="small", bufs=2) as sp,
            tc.tile_pool(name="psf", bufs=2, space="PSUM") as psf_pool,
            tc.tile_pool(name="pst", bufs=2, space="PSUM") as pst_pool,
        ):
            tmeg = pp.tile([128, L], BF16)
            LF = NB * 2 * WP
            f4a = pp.tile([128, LF], BF16)
            f4b = pp.tile([128, LF], BF16)
            w1t4_sb = pp.tile([C, C], BF16)
            b14_sb = pp.tile([C, 1], F32)
            l3_sb = pp.tile([128, C], BF16)
            wb_sb = pp.tile([C, 1], F32)
            g2 = pp.tile([128, C], BF16)

            nc.sync.dma_start(out=w1t4_sb[:, :], in_=w1t4_d.ap())
            nc.sync.dma_start(out=b14_sb[:, :], in_=b14_d.ap())
            nc.sync.dma_start(out=l3_sb[64:82, :], in_=l3_d.ap())
            nc.sync.dma_start(out=wb_sb[:, :], in_=wb_d.ap())

            nc.vector.memset(g2[:, :], 0.0)
            nc.vector.memset(g2[:, GCOL + 9 : GCOL + 10], 1.0)
            nc.vector.memset(tmeg[TPL : TPL + 10, 0:WP], 0.0)
            nc.vector.memset(tmeg[TPL : TPL + 10, 193 * WP : 194 * WP], 0.0)
            for f4 in (f4a, f4b):
                f4v = f4.rearrange("p (b r c) -> p b r c", r=2, c=WP)
                nc.vector.memset(f4v[:, :, :, 0:1], 0.0)
                nc.vector.memset(f4v[:, :, :, WP - 1 : WP], 0.0)

            for n in range(NS):
                f4 = f4a if n % 2 == 0 else f4b

                # -------- pass 1: x in (bf16, split DMA), maxpool, MM1+relu
                ntiles = H // XROWS  # 12
                xp_part = sp.tile([128, ntiles * 48], F32, tag="xp_part")
                for j in range(ntiles):
                    xb = xbf_pool.tile([128, XROWS * W], BF16, tag="xb")
                    xb3 = xb.rearrange("p (r c) -> p r c", c=W)
                    for hh in range(2):
                        nc.sync.dma_start(
                            out=xb3[64 * hh : 64 * hh + 64],
                            in_=x_d.ap()[
                                n, 64 * hh : 64 * hh + 64,
                                j * XROWS : (j + 1) * XROWS, :,
                            ],
                        )
                    # maxpool stage 1: X-reduce over contiguous 64 cols
                    nc.vector.tensor_reduce(
                        xp_part[:, j * 48 : (j + 1) * 48],
                        xb.rearrange("p (r kx c) -> p r kx c", kx=3, c=64),
                        axis=mybir.AxisListType.X,
                        op=mybir.AluOpType.max,
                    )
                    ps = psf_pool.tile([128, 1024], F32, tag="psf", name="psf")
                    for half in range(2):
                        for g in range(4):
                            yloc = half * 8 + 2 * g
                            nc.tensor.matmul(
                                ps[
                                    32 * g : 32 * g + 32,
                                    half * 512 : half * 512 + RROWS * W,
                                ],
                                w1t4_sb[:, 32 * g : 32 * g + 32],
                                xb[:, yloc * W : (yloc + 2) * W],
                                tile_position=(0, 32 * g),
                            )
                    dst = f4.rearrange("p (b r c) -> p b r c", r=2, c=WP)[
                        :, 2 * j : 2 * j + 2, :, 1 : 1 + W
                    ]
                    src = ps.rearrange("p (h f) -> p h f", h=2)[
                        :, :, : RROWS * W
                    ].rearrange("p h (r c) -> p h r c", c=W)
                    if j % 2 == 0:
                        nc.scalar.activation(
                            dst, src,
                            mybir.ActivationFunctionType.Relu,
                            bias=b14_sb[:, :],
                        )
                    else:
                        nc.vector.tensor_scalar(
                            dst, src, b14_sb[:, :], 0.0,
                            op0=mybir.AluOpType.add,
                            op1=mybir.AluOpType.max,
                        )

                # -------- maxpool stage 2 (two strided combines), then g
                xp_mid = sp.tile([128, 36], F32, tag="xp_mid")
                nc.vector.tensor_reduce(
                    xp_mid[:, :],
                    xp_part.rearrange("p (j r kx) -> p j kx r", r=16, kx=3),
                    axis=mybir.AxisListType.X,
                    op=mybir.AluOpType.max,
                )
                xp_r = sp.tile([128, 10], BF16, tag="xp_r")
                nc.vector.memset(xp_r[:, 9:10], 0.0)
                nc.vector.tensor_reduce(
                    xp_r[:, 0:9],
                    xp_mid.rearrange("p (ky s kx) -> p ky kx s", ky=3, kx=3),
                    axis=mybir.AxisListType.X,
                    op=mybir.AluOpType.max,
                )
                pg = pst_pool.tile([128, 1024], F32, tag="pst", name="pg")[:, :10]
                nc.tensor.matmul(pg[:, :], w1t4_sb[:, :], xp_r[:, :])
                nc.scalar.activation(
                    g2[:, GCOL : GCOL + 9],
                    pg[:, 0:9],
                    mybir.ActivationFunctionType.Relu,
                    bias=b14_sb[:, :],
                )

                # -------- MM2 / T-shift / MM3 in two half-plane stages so
                # stage-1 MM2 overlaps stage-0 MM3 (kills the phase valley)
                tpl = tmeg[TPL : TPL + 10, :]
                f4r = f4.rearrange("p (b f) -> p b f", f=2 * WP)
                tsb = tmeg[TSB : TSB + 18, :].rearrange("p (r c) -> p r c", c=WP)

                def mm2_block(b):
                    for gp in range(2):
                        pT = pst_pool.tile([128, 1024], F32, tag="pst", name="pT")
                        for q in range(2):
                            g = 2 * gp + q
                            nc.tensor.matmul(
                                pT[TPL : TPL + 10, q * 512 : q * 512 + 2 * WP],
                                g2[32 * g : 32 * g + 32, GCOL : GCOL + 10],
                                f4r[32 * g : 32 * g + 32, b, :],
                                tile_position=(32 * g, TPL),
                            )
                        py0 = 8 * b + 4 * gp + 1
                        dst = tpl[:, py0 * WP : (py0 + 4) * WP]
                        src = pT.rearrange("p (q f) -> p q f", q=2)[
                            TPL : TPL + 10, :, : 2 * WP
                        ]
                        nc.vector.tensor_copy(dst, src)

                def tshift(stage):
                    CH0, CH1 = 19040, 18820
                    for t in range(18):
                        src_row = TPL + (t if t < 9 else 9)
                        d = DELTAS[t % 9]
                        if stage == 0:
                            lo = max(0, -d)
                            hi = lo + CH0
                        else:
                            hi = L - max(0, d)
                            lo = hi - CH1
                        nc.scalar.dma_start(
                            out=tmeg[TSB + t : TSB + t + 1, lo:hi],
                            in_=tmeg[src_row : src_row + 1, lo + d : hi + d],
                        )

                def mm3_tile(j):
                    ot = out_pool.tile([128, XROWS * W], BF16, tag="ot")
                    for half in range(4):
                        po = pst_pool.tile([128, 1024], F32, tag="pst", name="po")
                        for q in range(2):
                            y0 = j * XROWS + (half * 2 + q) * RROWS
                            nc.tensor.matmul(
                                po[:, q * 512 : q * 512 + RROWS * W],
                                l3_sb[64:82, :],
                                tsb[:, y0 + 1 : y0 + 1 + RROWS, 1 : 1 + W],
                            )
                        dst = ot[
                            :, half * 2 * RROWS * W : (half + 1) * 2 * RROWS * W
                        ]
                        src = po.rearrange("p (q f) -> p q f", q=2)[
                            :, :, : RROWS * W
                        ]
                        if (j + half) % 2 == 0:
                            nc.scalar.activation(
                                dst, src,
                                mybir.ActivationFunctionType.Identity,
                                bias=wb_sb[:, :],
                            )
                        else:
                            nc.vector.tensor_scalar(
                                dst, src, wb_sb[:, :], None,
                                op0=mybir.AluOpType.add,
                            )
                    ot3 = ot.rearrange("p (r c) -> p r c", c=W)
                    for hh in range(2):
                        nc.scalar.dma_start(
                            out=out_d.ap()[
                                n, 64 * hh : 64 * hh + 64,
                                j * XROWS : (j + 1) * XROWS, :,
                            ],
                            in_=ot3[64 * hh : 64 * hh + 64],
                        )

                for b in range(14):
                    mm2_block(b)
                tshift(0)
                for b in range(14, NB):
                    mm2_block(b)
                for j in range(ntiles // 2):
                    mm3_tile(j)
                tshift(1)
                for j in range(ntiles // 2, ntiles):
                    mm3_tile(j)
    return nc


_CACHE = {}


def _get_nc():
    if "nc" not in _CACHE:
        nc = bacc.Bacc(
            "TRN2", target_bir_lowering=False, debug=False, num_devices=N_CORES
        )
        build(nc)
        nc.compile()
        _CACHE["nc"] = nc
    return _CACHE["nc"]


def make_in_maps(x, W1, b1, Wk, bk, b_adap, Wf, bf):
    x = np.asarray(x, dtype=np.float32)
    W1 = np.asarray(W1, dtype=np.float32)
    b1 = np.asarray(b1, dtype=np.float32)
    Wk = np.asarray(Wk, dtype=np.float32)
    bk = np.asarray(bk, dtype=np.float32)
    b_adap = np.asarray(b_adap, dtype=np.float32)
    Wf = np.asarray(Wf, dtype=np.float32)
    bf = np.asarray(bf, dtype=np.float32)

    u = Wf @ Wk
    v = Wf @ bk
    w = Wf @ b_adap + bf
    l3 = np.ascontiguousarray(np.stack([u] * 9 + [v] * 9).astype(ml_dtypes.bfloat16))
    w1t4 = np.ascontiguousarray(np.tile(W1.T, (1, 4)).astype(ml_dtypes.bfloat16))
    b14 = np.ascontiguousarray(np.tile(b1, 4)[:, None].astype(np.float32))
    wbc = np.ascontiguousarray(w[:, None].astype(np.float32))
    xb = np.ascontiguousarray(x.astype(ml_dtypes.bfloat16))

    in_maps = []
    for i in range(N_CORES):
        in_maps.append(
            {
                "x": xb[i * NS : (i + 1) * NS],
                "w1t4": w1t4,
                "b14": b14,
                "l3": l3,
                "wb": wbc,
            }
        )
    return in_maps


def kernel(x, W1, b1, Wk, bk, b_adap, Wf, bf):
    nc = _get_nc()
    in_maps = make_in_maps(x, W1, b1, Wk, bk, b_adap, Wf, bf)
    res = run_bass_kernel_spmd(nc, in_maps, list(range(N_CORES)))
    return np.concatenate(
        [np.asarray(res.results[i]["out"]).astype(np.float32) for i in range(N_CORES)],
        axis=0,
    )
